# revision 38
# baseline (speedup 1.0000x reference)
"""Trainium2 Bass kernel for a dense transformer layer (attention + FFN).

Sharding: 8 shards = (batch b, sequence half) pairs. Each core computes the
full K/V projections for its batch (2x redundant) and Q/attention/FFN for its
1024-token query slice. No cross-core communication.

On-device layout is feature-major (transposed): activations live as
[feature, token] so every matmul is lhsT.T @ rhs with natural weight layouts.
Compute in bf16 on the TensorEngine with fp32 PSUM accumulation; residuals in
fp32.
"""

import numpy as np
import ml_dtypes

B, S, D = 4, 2048, 1024
H, DH, F = 16, 64, 4096
P = 128
NCORES = 8
SQ = B * S // NCORES  # 1024 query tokens per core
DC = D // P  # 8 feature chunks
FC = F // P  # 32 ffn chunks
SKC = S // P  # 16 key chunks
NPAIR = H // 2  # 8 head pairs (2 heads per 128-feature chunk)

BF16 = ml_dtypes.bfloat16

_CACHE = {}


def _build_program():
    import concourse.mybir as mybir
    import concourse.tile as tile
    from concourse import bacc

    f32 = mybir.dt.float32
    bf16 = mybir.dt.bfloat16
    AF = mybir.ActivationFunctionType

    nc = bacc.Bacc("TRN2", target_bir_lowering=False, debug=False, num_devices=NCORES)

    xT_d = nc.dram_tensor("xT", [P, DC, S], bf16, kind="ExternalInput")
    xqT_d = nc.dram_tensor("xqT", [P, DC, SQ], bf16, kind="ExternalInput")
    xres_d = nc.dram_tensor("xres", [P, DC, SQ], f32, kind="ExternalInput")
    wq_d = nc.dram_tensor("wq", [P, DC, D], bf16, kind="ExternalInput")
    wk_d = nc.dram_tensor("wk", [P, DC, D], bf16, kind="ExternalInput")
    wv_d = nc.dram_tensor("wv", [P, DC, D], bf16, kind="ExternalInput")
    wo_d = nc.dram_tensor("wo", [P, DC, D], bf16, kind="ExternalInput")
    w1_d = nc.dram_tensor("w1", [FC, P, DC, P], bf16, kind="ExternalInput")
    w2_d = nc.dram_tensor("w2", [DC, P, FC, P], bf16, kind="ExternalInput")
    bq_d = nc.dram_tensor("bq", [P, DC], f32, kind="ExternalInput")
    bk_d = nc.dram_tensor("bk", [P, DC], f32, kind="ExternalInput")
    bvb_d = nc.dram_tensor("bvb", [P, D], bf16, kind="ExternalInput")
    b1_d = nc.dram_tensor("b1", [P, FC], f32, kind="ExternalInput")
    b2_d = nc.dram_tensor("b2", [P, DC], f32, kind="ExternalInput")
    outT_d = nc.dram_tensor("outT", [P, DC, SQ], f32, kind="ExternalOutput")

    with tile.TileContext(nc) as tc:
        with (
            tc.tile_pool(name="psA", bufs=2, space="PSUM") as psA,
            tc.tile_pool(name="psS", bufs=2, space="PSUM") as psS,
            tc.tile_pool(name="psC", bufs=2, space="PSUM") as psC,
            tc.tile_pool(name="biasp", bufs=1) as biasp,
        ):
            bq_sb = biasp.tile([P, DC], f32)
            bk_sb = biasp.tile([P, DC], f32)
            b1_sb = biasp.tile([P, FC], f32)
            b2_sb = biasp.tile([P, DC], f32)
            nc.sync.dma_start(bq_sb[:], bq_d[:])
            nc.sync.dma_start(bk_sb[:], bk_d[:])
            nc.sync.dma_start(b1_sb[:], b1_d[:])
            nc.sync.dma_start(b2_sb[:], b2_d[:])

            with (
                tc.tile_pool(name="ctxp", bufs=1) as ctxp,
                tc.tile_pool(name="wop", bufs=1) as wop,
            ):
                ctxT_sb = ctxp.tile([P, DC, SQ], bf16)
                # attn_out accumulated across pairs (O-projection folded into
                # phase B); later overwritten in place with bf16(y) for the FFN
                acc_sb = ctxp.tile([P, DC, SQ], bf16)
                wo_sb = wop.tile([P, DC, D], bf16)

                # ---------------- Phase A+B: QKV projections + attention ----
                with (
                    tc.tile_pool(name="abp", bufs=1) as abp,
                    tc.tile_pool(name="wvp", bufs=1) as wvp,
                    tc.tile_pool(name="ws", bufs=3) as ws,
                    tc.tile_pool(name="ep", bufs=6) as ep,
                    tc.tile_pool(name="ktp", bufs=2) as ktp,
                    tc.tile_pool(name="qtp", bufs=2) as qtp,
                    tc.tile_pool(name="rp", bufs=2) as rp,
                    tc.tile_pool(name="rbp", bufs=2) as rbp,
                    tc.tile_pool(name="stp", bufs=2) as stp,
                ):
                    # x^T in 4 column-chunk tiles so V/K matmuls start after the
                    # first chunk lands rather than after the full 4MB DMA.
                    xTs = [
                        abp.tile([P, DC, 512], bf16, tag=f"xT{c}", name=f"xT{c}")
                        for c in range(4)
                    ]
                    wvs = [
                        wvp.tile([P, DC, 512], bf16, tag=f"wv{c}", name=f"wv{c}")
                        for c in range(2)
                    ]
                    bvb_sb = abp.tile([P, D], bf16)
                    xqT_sb = abp.tile([P, DC, SQ], bf16)
                    # DMA order matters at startup: the first V matmuls need
                    # xTs[0] + wvs[0] only.
                    # k-chunk interleaved so V's first matmuls start after
                    # ~256KB arrives instead of the full 2MB
                    for k in range(DC):
                        nc.sync.dma_start(
                            xTs[0][:, k : k + 1, :], xT_d[:, k : k + 1, 0:512]
                        )
                        nc.gpsimd.dma_start(
                            wvs[0][:, k : k + 1, :], wv_d[:, k : k + 1, 0:512]
                        )
                    nc.sync.dma_start(bvb_sb[:], bvb_d[:])
                    for c in range(1, 4):
                        nc.sync.dma_start(xTs[c][:], xT_d[:, :, c * 512 : (c + 1) * 512])
                    nc.sync.dma_start(wvs[1][:], wv_d[:, :, 512:1024])
                    nc.sync.dma_start(xqT_sb[:], xqT_d[:])
                    nc.sync.dma_start(wo_sb[:], wo_d[:])

                    # V projection, token-major: v[sk, dv] (+ ones column per head)
                    v_sb = abp.tile([P, SKC, H, DH + 1], bf16)
                    nc.vector.memset(v_sb[:, :, :, DH : DH + 1], 1.0)

                    def emit_v(nv, sks, h0=0, h1=8):
                        nh = h1 - h0
                        for sk in sks:
                            xt = xTs[sk // 4]
                            co = (sk % 4) * P
                            ps = psA.tile([P, 512], f32, tag="ps")
                            for k in range(DC):
                                nc.tensor.matmul(
                                    ps[:, : nh * DH],
                                    xt[:, k, co : co + P],
                                    wvs[nv][:, k, h0 * DH : h1 * DH],
                                    start=(k == 0),
                                    stop=(k == DC - 1),
                                )
                            nc.vector.tensor_add(
                                v_sb[:, sk, nv * 8 + h0 : nv * 8 + h1, 0:DH],
                                ps[:, : nh * DH].rearrange("p (h d) -> p h d", h=nh),
                                bvb_sb[
                                    :, nv * 512 + h0 * DH : nv * 512 + h1 * DH
                                ].rearrange("p (h d) -> p h d", h=nh),
                            )


                    # V(nv=1) matmul chunks, spread across the ACT-bound
                    # attention segments of pairs 1-3 (needed from pair 4 on).
                    V1_SPREAD = {
                        (0, 1): range(0, 2),
                        (1, 0): range(2, 5), (1, 1): range(5, 8),
                        (2, 0): range(8, 11), (2, 1): range(11, 14),
                        (3, 0): range(14, 16),
                    }

                    emit_v(0, range(SKC))
                    for p in range(NPAIR):
                        # K/Q for this pair only; double-buffered so pair p+1's
                        # projections overlap pair p's attention
                        kt = ktp.tile([P, S], bf16, tag="kt", name="kt")
                        qt = qtp.tile([P, SQ], bf16, tag="qt", name="qt")
                        # K projection for feature chunk p (heads 2p, 2p+1)
                        wkt = ws.tile([P, DC, P], bf16, tag="wchunk")
                        nc.sync.dma_start(wkt[:], wk_d[:, :, p * P : (p + 1) * P])
                        for n in range(S // 512):
                            ps = psA.tile([P, 512], f32)
                            for k in range(DC):
                                nc.tensor.matmul(
                                    ps,
                                    wkt[:, k, :],
                                    xTs[n][:, k, :],
                                    start=(k == 0),
                                    stop=(k == DC - 1),
                                )
                            nc.vector.tensor_scalar_add(
                                kt[:, n * 512 : (n + 1) * 512],
                                ps,
                                bk_sb[:, p : p + 1],
                            )
                        # Q projection for feature chunk p (pre-scaled weights)
                        wqt = ws.tile([P, DC, P], bf16, tag="wchunk")
                        nc.sync.dma_start(wqt[:], wq_d[:, :, p * P : (p + 1) * P])
                        for n in range(SQ // 512):
                            ps = psA.tile([P, 512], f32)
                            for k in range(DC):
                                nc.tensor.matmul(
                                    ps,
                                    wqt[:, k, :],
                                    xqT_sb[:, k, n * 512 : (n + 1) * 512],
                                    start=(k == 0),
                                    stop=(k == DC - 1),
                                )
                            nc.vector.tensor_scalar_add(
                                qt[:, n * 512 : (n + 1) * 512],
                                ps,
                                bq_sb[:, p : p + 1],
                            )

                        # Attention for head pair (2p, 2p+1)
                        for sqn in range(SQ // 512):
                            pc0 = psC.tile([P, 512], f32, tag="pc")
                            pc1 = psC.tile([P, 512], f32, tag="pc")
                            for sk in range(SKC):
                                ss = psS.tile([P, 1024], f32)
                                nc.tensor.matmul(
                                    ss[:, 0:512],
                                    kt[0:64, sk * P : (sk + 1) * P],
                                    qt[0:64, sqn * 512 : (sqn + 1) * 512],
                                    start=True,
                                    stop=True,
                                )
                                nc.tensor.matmul(
                                    ss[:, 512:1024],
                                    kt[64:128, sk * P : (sk + 1) * P],
                                    qt[64:128, sqn * 512 : (sqn + 1) * 512],
                                    start=True,
                                    stop=True,
                                )
                                E = ep.tile([P, 1024], bf16)
                                nc.scalar.activation(E, ss, AF.Exp)
                                nc.tensor.matmul(
                                    pc0[:65],
                                    v_sb[:, sk, 2 * p, :],
                                    E[:, 0:512],
                                    start=(sk == 0),
                                    stop=(sk == SKC - 1),
                                )
                                nc.tensor.matmul(
                                    pc1[:65],
                                    v_sb[:, sk, 2 * p + 1, :],
                                    E[:, 512:1024],
                                    start=(sk == 0),
                                    stop=(sk == SKC - 1),
                                )
                            # softmax normalization: ctx / rowsum (approx recip
                            # is ~18 correct bits, plenty for a softmax denom)
                            s0 = rp.tile([1, 512], f32, tag="s")
                            nc.vector.tensor_copy(s0, pc0[64:65, :])
                            r0 = rp.tile([1, 512], f32, tag="r")
                            nc.vector.reciprocal_approx_fast(r0, s0)
                            rb0 = rbp.tile([64, 512], f32, tag="rb")
                            nc.gpsimd.partition_broadcast(rb0, r0)
                            nc.vector.tensor_mul(
                                ctxT_sb[0:64, p, sqn * 512 : (sqn + 1) * 512],
                                pc0[0:64, :],
                                rb0,
                            )
                            s1 = rp.tile([1, 512], f32, tag="s")
                            nc.vector.tensor_copy(s1, pc1[64:65, :])
                            r1 = rp.tile([1, 512], f32, tag="r")
                            nc.vector.reciprocal_approx_fast(r1, s1)
                            rb1 = rbp.tile([64, 512], f32, tag="rb")
                            nc.gpsimd.partition_broadcast(rb1, r1)
                            nc.vector.tensor_mul(
                                ctxT_sb[64:128, p, sqn * 512 : (sqn + 1) * 512],
                                pc1[0:64, :],
                                rb1,
                            )
                            if (p, sqn) in V1_SPREAD:
                                emit_v(1, V1_SPREAD[(p, sqn)])

                # ---------------- Phase C: O projection + residual ----------
                with (
                    tc.tile_pool(name="ytp", bufs=1) as ytp,
                ):
                    yT_sb = ytp.tile([P, DC, SQ], f32)
                    with tc.tile_pool(name="xrp", bufs=3) as xrp:
                        for m in range(DC):
                            xr = xrp.tile([P, SQ], f32, tag="xr")
                            nc.sync.dma_start(xr[:], xres_d[:, m, :])
                            for n in range(SQ // 512):
                                ps = psA.tile([P, 512], f32)
                                for k in range(DC):
                                    nc.tensor.matmul(
                                        ps,
                                        wo_sb[:, k, m * P : (m + 1) * P],
                                        ctxT_sb[:, k, n * 512 : (n + 1) * 512],
                                        start=(k == 0),
                                        stop=(k == DC - 1),
                                    )
                                nc.vector.tensor_add(
                                    yT_sb[:, m, n * 512 : (n + 1) * 512],
                                    ps,
                                    xr[:, n * 512 : (n + 1) * 512],
                                )
                                # bf16(y) into acc_sb: the FFN reads it as its
                                # input activation
                                nc.scalar.activation(
                                    acc_sb[:, m, n * 512 : (n + 1) * 512],
                                    yT_sb[:, m, n * 512 : (n + 1) * 512],
                                    AF.Copy,
                                )

                    # ---------------- Phase D: FFN layer 1 + gelu -----------
                    with tc.tile_pool(name="htp", bufs=1) as htp:
                        hT_sb = htp.tile([P, FC, SQ], bf16)
                        with (
                            tc.tile_pool(name="w2s", bufs=2) as w2s,
                            tc.tile_pool(name="outp", bufs=4) as outp,
                            tc.tile_pool(name="w1s", bufs=3) as w1s,
                        ):
                            w2t0 = w2s.tile([P, FC, P], bf16, tag="w2c", name="w2t0")
                            nc.sync.dma_start(w2t0[:], w2_d[0])
                            for m in range(FC):
                                w1t = w1s.tile([P, DC, P], bf16, tag="w1c")
                                nc.sync.dma_start(w1t[:], w1_d[m])
                                for n in range(SQ // 512):
                                    ps = psA.tile([P, 512], f32)
                                    for k in range(DC):
                                        nc.tensor.matmul(
                                            ps,
                                            w1t[:, k, :],
                                            acc_sb[:, k, n * 512 : (n + 1) * 512],
                                            start=(k == 0),
                                            stop=(k == DC - 1),
                                        )
                                    nc.scalar.activation(
                                        hT_sb[:, m, n * 512 : (n + 1) * 512],
                                        ps,
                                        AF.Gelu,
                                        bias=b1_sb[:, m : m + 1],
                                    )

                            # ------------ Phase E: FFN layer 2 + residual ---
                            for m in range(DC):
                                if m == 0:
                                    w2t = w2t0
                                else:
                                    w2t = w2s.tile([P, FC, P], bf16, tag="w2c")
                                    nc.sync.dma_start(w2t[:], w2_d[m])
                                for n in range(SQ // 512):
                                    ps = psA.tile([P, 512], f32)
                                    for k in range(FC):
                                        nc.tensor.matmul(
                                            ps,
                                            w2t[:, k, :],
                                            hT_sb[:, k, n * 512 : (n + 1) * 512],
                                            start=(k == 0),
                                            stop=(k == FC - 1),
                                        )
                                    ot = outp.tile([P, 512], f32, tag="ot")
                                    nc.scalar.activation(
                                        ot, ps, AF.Identity, bias=b2_sb[:, m : m + 1]
                                    )
                                    nc.vector.tensor_add(
                                        ot, ot, yT_sb[:, m, n * 512 : (n + 1) * 512]
                                    )
                                    nc.sync.dma_start(
                                        outT_d[:, m, n * 512 : (n + 1) * 512], ot
                                    )

    nc.compile()
    return nc


def _get_program():
    if "nc" not in _CACHE:
        _CACHE["nc"] = _build_program()
    return _CACHE["nc"]


def _wlayout(W):
    # [D_in, D_out] -> [P, D_in//P, D_out]
    return np.ascontiguousarray(
        W.reshape(W.shape[0] // P, P, W.shape[1]).transpose(1, 0, 2)
    )


def _blayout(b):
    # [D] -> [P, D//P]
    return np.ascontiguousarray(b.reshape(b.shape[0] // P, P).T)


def prepare_in_maps(x, Wq, bq, Wk, bk, Wv, bv, Wo, bo, W1, b1, W2, b2):
    x = np.asarray(x, np.float32)
    Wq = np.asarray(Wq, np.float32)
    bq = np.asarray(bq, np.float32)
    Wk = np.asarray(Wk, np.float32)
    bk = np.asarray(bk, np.float32)
    Wv = np.asarray(Wv, np.float32)
    bv = np.asarray(bv, np.float32)
    Wo = np.asarray(Wo, np.float32)
    bo = np.asarray(bo, np.float32)
    W1 = np.asarray(W1, np.float32)
    b1 = np.asarray(b1, np.float32)
    W2 = np.asarray(W2, np.float32)
    b2 = np.asarray(b2, np.float32)

    scale = DH ** -0.5
    shared = {
        "wq": _wlayout(Wq * scale).astype(BF16),
        "wk": _wlayout(Wk).astype(BF16),
        "wv": _wlayout(Wv).astype(BF16),
        "wo": _wlayout(Wo).astype(BF16),
        "w1": np.ascontiguousarray(
            W1.reshape(DC, P, FC, P).transpose(2, 1, 0, 3)
        ).astype(BF16),
        "w2": np.ascontiguousarray(
            W2.reshape(FC, P, DC, P).transpose(2, 1, 0, 3)
        ).astype(BF16),
        "bq": _blayout(bq * scale),
        "bk": _blayout(bk),
        "bvb": np.ascontiguousarray(np.broadcast_to(bv, (P, D))).astype(BF16),
        "b1": _blayout(b1),
        "b2": _blayout(b2),
    }

    in_maps = []
    for c in range(NCORES):
        b_idx, half = divmod(c, 2)
        xb = x[b_idx]  # [S, D]
        xbT = xb.T  # [D, S]
        xT = np.ascontiguousarray(
            xbT.reshape(DC, P, S).transpose(1, 0, 2)
        ).astype(BF16)
        xqT = np.ascontiguousarray(
            xbT[:, half * SQ : (half + 1) * SQ]
            .reshape(DC, P, SQ)
            .transpose(1, 0, 2)
        ).astype(BF16)
        xres = np.ascontiguousarray(
            (xbT[:, half * SQ : (half + 1) * SQ] + bo[:, None])
            .reshape(DC, P, SQ)
            .transpose(1, 0, 2)
        ).astype(np.float32)
        in_maps.append(dict(shared, xT=xT, xqT=xqT, xres=xres))
    return in_maps


def assemble_out(results):
    out = np.empty((B, S, D), np.float32)
    for c in range(NCORES):
        b_idx, half = divmod(c, 2)
        outT = results[c]["outT"]  # [P, DC, SQ]
        out[b_idx, half * SQ : (half + 1) * SQ] = (
            outT.transpose(1, 0, 2).reshape(D, SQ).T
        )
    return out


def kernel(**inputs):
    from concourse.bass_utils import run_bass_kernel_spmd

    in_maps = prepare_in_maps(**inputs)
    nc = _get_program()
    res = run_bass_kernel_spmd(nc, in_maps, core_ids=list(range(NCORES)))
    return assemble_out(res.results)


# revision 39
# speedup vs baseline: 1.0039x; 1.0039x over previous
"""Trainium2 Bass kernel for a dense transformer layer (attention + FFN).

Sharding: 8 shards = (batch b, sequence half) pairs. Each core computes the
full K/V projections for its batch (2x redundant) and Q/attention/FFN for its
1024-token query slice. No cross-core communication.

On-device layout is feature-major (transposed): activations live as
[feature, token] so every matmul is lhsT.T @ rhs with natural weight layouts.
Compute in bf16 on the TensorEngine with fp32 PSUM accumulation; residuals in
fp32.
"""

import numpy as np
import ml_dtypes

B, S, D = 4, 2048, 1024
H, DH, F = 16, 64, 4096
P = 128
NCORES = 8
SQ = B * S // NCORES  # 1024 query tokens per core
DC = D // P  # 8 feature chunks
FC = F // P  # 32 ffn chunks
SKC = S // P  # 16 key chunks
NPAIR = H // 2  # 8 head pairs (2 heads per 128-feature chunk)

BF16 = ml_dtypes.bfloat16

_CACHE = {}


def _build_program():
    import concourse.mybir as mybir
    import concourse.tile as tile
    from concourse import bacc

    f32 = mybir.dt.float32
    bf16 = mybir.dt.bfloat16
    AF = mybir.ActivationFunctionType

    nc = bacc.Bacc("TRN2", target_bir_lowering=False, debug=False, num_devices=NCORES)

    xT_d = nc.dram_tensor("xT", [P, DC, S], bf16, kind="ExternalInput")
    xqT_d = nc.dram_tensor("xqT", [P, DC, SQ], bf16, kind="ExternalInput")
    xres_d = nc.dram_tensor("xres", [P, DC, SQ], f32, kind="ExternalInput")
    wq_d = nc.dram_tensor("wq", [P, DC, D], bf16, kind="ExternalInput")
    wk_d = nc.dram_tensor("wk", [P, DC, D], bf16, kind="ExternalInput")
    wv_d = nc.dram_tensor("wv", [P, DC, D], bf16, kind="ExternalInput")
    wo_d = nc.dram_tensor("wo", [P, DC, D], bf16, kind="ExternalInput")
    w1_d = nc.dram_tensor("w1", [FC, P, DC, P], bf16, kind="ExternalInput")
    w2_d = nc.dram_tensor("w2", [DC, P, FC, P], bf16, kind="ExternalInput")
    bq_d = nc.dram_tensor("bq", [P, DC], f32, kind="ExternalInput")
    bk_d = nc.dram_tensor("bk", [P, DC], f32, kind="ExternalInput")
    bvb_d = nc.dram_tensor("bvb", [P, D], bf16, kind="ExternalInput")
    b1_d = nc.dram_tensor("b1", [P, FC], f32, kind="ExternalInput")
    b2_d = nc.dram_tensor("b2", [P, DC], f32, kind="ExternalInput")
    outT_d = nc.dram_tensor("outT", [P, DC, SQ], f32, kind="ExternalOutput")

    with tile.TileContext(nc) as tc:
        with (
            tc.tile_pool(name="psA", bufs=2, space="PSUM") as psA,
            tc.tile_pool(name="psS", bufs=2, space="PSUM") as psS,
            tc.tile_pool(name="psC", bufs=2, space="PSUM") as psC,
            tc.tile_pool(name="biasp", bufs=1) as biasp,
        ):
            bq_sb = biasp.tile([P, DC], f32)
            bk_sb = biasp.tile([P, DC], f32)
            b1_sb = biasp.tile([P, FC], f32)
            b2_sb = biasp.tile([P, DC], f32)
            nc.sync.dma_start(bq_sb[:], bq_d[:])
            nc.sync.dma_start(bk_sb[:], bk_d[:])
            nc.sync.dma_start(b1_sb[:], b1_d[:])
            nc.sync.dma_start(b2_sb[:], b2_d[:])

            with (
                tc.tile_pool(name="ctxp", bufs=1) as ctxp,
                tc.tile_pool(name="wop", bufs=1) as wop,
            ):
                ctxT_sb = ctxp.tile([P, DC, SQ], bf16)
                # attn_out accumulated across pairs (O-projection folded into
                # phase B); later overwritten in place with bf16(y) for the FFN
                acc_sb = ctxp.tile([P, DC, SQ], bf16)
                wo_sb = wop.tile([P, DC, D], bf16)

                # ---------------- Phase A+B: QKV projections + attention ----
                with (
                    tc.tile_pool(name="abp", bufs=1) as abp,
                    tc.tile_pool(name="wvp", bufs=1) as wvp,
                    tc.tile_pool(name="ws", bufs=5) as ws,
                    tc.tile_pool(name="ep", bufs=6) as ep,
                    tc.tile_pool(name="ktp", bufs=3) as ktp,
                    tc.tile_pool(name="qtp", bufs=3) as qtp,
                    tc.tile_pool(name="rp", bufs=2) as rp,
                    tc.tile_pool(name="rbp", bufs=2) as rbp,
                    tc.tile_pool(name="stp", bufs=2) as stp,
                ):
                    # x^T in 4 column-chunk tiles so V/K matmuls start after the
                    # first chunk lands rather than after the full 4MB DMA.
                    xTs = [
                        abp.tile([P, DC, 512], bf16, tag=f"xT{c}", name=f"xT{c}")
                        for c in range(4)
                    ]
                    wvs = [
                        wvp.tile([P, DC, 512], bf16, tag=f"wv{c}", name=f"wv{c}")
                        for c in range(2)
                    ]
                    bvb_sb = abp.tile([P, D], bf16)
                    xqT_sb = abp.tile([P, DC, SQ], bf16)
                    # DMA order matters at startup: the first V matmuls need
                    # xTs[0] + wvs[0] only.
                    # k-chunk interleaved so V's first matmuls start after
                    # ~256KB arrives instead of the full 2MB
                    for k in range(DC):
                        nc.sync.dma_start(
                            xTs[0][:, k : k + 1, :], xT_d[:, k : k + 1, 0:512]
                        )
                        nc.gpsimd.dma_start(
                            wvs[0][:, k : k + 1, :], wv_d[:, k : k + 1, 0:512]
                        )
                    nc.sync.dma_start(bvb_sb[:], bvb_d[:])
                    for c in range(1, 4):
                        nc.sync.dma_start(xTs[c][:], xT_d[:, :, c * 512 : (c + 1) * 512])
                    nc.sync.dma_start(wvs[1][:], wv_d[:, :, 512:1024])
                    nc.sync.dma_start(xqT_sb[:], xqT_d[:])
                    nc.sync.dma_start(wo_sb[:], wo_d[:])

                    # V projection, token-major: v[sk, dv] (+ ones column per head)
                    v_sb = abp.tile([P, SKC, H, DH + 1], bf16)
                    nc.vector.memset(v_sb[:, :, :, DH : DH + 1], 1.0)

                    def emit_v(nv, sks, h0=0, h1=8):
                        nh = h1 - h0
                        for sk in sks:
                            xt = xTs[sk // 4]
                            co = (sk % 4) * P
                            ps = psA.tile([P, 512], f32, tag="ps")
                            for k in range(DC):
                                nc.tensor.matmul(
                                    ps[:, : nh * DH],
                                    xt[:, k, co : co + P],
                                    wvs[nv][:, k, h0 * DH : h1 * DH],
                                    start=(k == 0),
                                    stop=(k == DC - 1),
                                )
                            nc.vector.tensor_add(
                                v_sb[:, sk, nv * 8 + h0 : nv * 8 + h1, 0:DH],
                                ps[:, : nh * DH].rearrange("p (h d) -> p h d", h=nh),
                                bvb_sb[
                                    :, nv * 512 + h0 * DH : nv * 512 + h1 * DH
                                ].rearrange("p (h d) -> p h d", h=nh),
                            )


                    # V(nv=1) matmul chunks, spread across the ACT-bound
                    # attention segments of pairs 1-3 (needed from pair 4 on).
                    V1_SPREAD = {
                        (0, 1): range(0, 2),
                        (1, 0): range(2, 5), (1, 1): range(5, 8),
                        (2, 0): range(8, 11), (2, 1): range(11, 14),
                        (3, 0): range(14, 16),
                    }

                    emit_v(0, range(SKC))
                    for p in range(NPAIR):
                        # K/Q for this pair only; double-buffered so pair p+1's
                        # projections overlap pair p's attention
                        kt = ktp.tile([P, S], bf16, tag="kt", name="kt")
                        qt = qtp.tile([P, SQ], bf16, tag="qt", name="qt")
                        # K projection for feature chunk p (heads 2p, 2p+1)
                        wkt = ws.tile([P, DC, P], bf16, tag="wchunk")
                        nc.sync.dma_start(wkt[:], wk_d[:, :, p * P : (p + 1) * P])
                        for n in range(S // 512):
                            ps = psA.tile([P, 512], f32)
                            for k in range(DC):
                                nc.tensor.matmul(
                                    ps,
                                    wkt[:, k, :],
                                    xTs[n][:, k, :],
                                    start=(k == 0),
                                    stop=(k == DC - 1),
                                )
                            nc.vector.tensor_scalar_add(
                                kt[:, n * 512 : (n + 1) * 512],
                                ps,
                                bk_sb[:, p : p + 1],
                            )
                        # Q projection for feature chunk p (pre-scaled weights)
                        wqt = ws.tile([P, DC, P], bf16, tag="wchunk")
                        nc.sync.dma_start(wqt[:], wq_d[:, :, p * P : (p + 1) * P])
                        for n in range(SQ // 512):
                            ps = psA.tile([P, 512], f32)
                            for k in range(DC):
                                nc.tensor.matmul(
                                    ps,
                                    wqt[:, k, :],
                                    xqT_sb[:, k, n * 512 : (n + 1) * 512],
                                    start=(k == 0),
                                    stop=(k == DC - 1),
                                )
                            nc.vector.tensor_scalar_add(
                                qt[:, n * 512 : (n + 1) * 512],
                                ps,
                                bq_sb[:, p : p + 1],
                            )

                        # Attention for head pair (2p, 2p+1)
                        for sqn in range(SQ // 512):
                            pc0 = psC.tile([P, 512], f32, tag="pc")
                            pc1 = psC.tile([P, 512], f32, tag="pc")
                            for sk in range(SKC):
                                ss = psS.tile([P, 1024], f32)
                                nc.tensor.matmul(
                                    ss[:, 0:512],
                                    kt[0:64, sk * P : (sk + 1) * P],
                                    qt[0:64, sqn * 512 : (sqn + 1) * 512],
                                    start=True,
                                    stop=True,
                                )
                                nc.tensor.matmul(
                                    ss[:, 512:1024],
                                    kt[64:128, sk * P : (sk + 1) * P],
                                    qt[64:128, sqn * 512 : (sqn + 1) * 512],
                                    start=True,
                                    stop=True,
                                )
                                E = ep.tile([P, 1024], bf16)
                                nc.scalar.activation(E, ss, AF.Exp)
                                nc.tensor.matmul(
                                    pc0[:65],
                                    v_sb[:, sk, 2 * p, :],
                                    E[:, 0:512],
                                    start=(sk == 0),
                                    stop=(sk == SKC - 1),
                                )
                                nc.tensor.matmul(
                                    pc1[:65],
                                    v_sb[:, sk, 2 * p + 1, :],
                                    E[:, 512:1024],
                                    start=(sk == 0),
                                    stop=(sk == SKC - 1),
                                )
                            # softmax normalization: ctx / rowsum (approx recip
                            # is ~18 correct bits, plenty for a softmax denom)
                            s0 = rp.tile([1, 512], f32, tag="s")
                            nc.vector.tensor_copy(s0, pc0[64:65, :])
                            r0 = rp.tile([1, 512], f32, tag="r")
                            nc.vector.reciprocal_approx_fast(r0, s0)
                            rb0 = rbp.tile([64, 512], f32, tag="rb")
                            nc.gpsimd.partition_broadcast(rb0, r0)
                            nc.vector.tensor_mul(
                                ctxT_sb[0:64, p, sqn * 512 : (sqn + 1) * 512],
                                pc0[0:64, :],
                                rb0,
                            )
                            s1 = rp.tile([1, 512], f32, tag="s")
                            nc.vector.tensor_copy(s1, pc1[64:65, :])
                            r1 = rp.tile([1, 512], f32, tag="r")
                            nc.vector.reciprocal_approx_fast(r1, s1)
                            rb1 = rbp.tile([64, 512], f32, tag="rb")
                            nc.gpsimd.partition_broadcast(rb1, r1)
                            nc.vector.tensor_mul(
                                ctxT_sb[64:128, p, sqn * 512 : (sqn + 1) * 512],
                                pc1[0:64, :],
                                rb1,
                            )
                            if (p, sqn) in V1_SPREAD:
                                emit_v(1, V1_SPREAD[(p, sqn)])

                # ---------------- Phase C: O projection + residual ----------
                with (
                    tc.tile_pool(name="ytp", bufs=1) as ytp,
                ):
                    yT_sb = ytp.tile([P, DC, SQ], f32)
                    with tc.tile_pool(name="xrp", bufs=3) as xrp:
                        for m in range(DC):
                            xr = xrp.tile([P, SQ], f32, tag="xr")
                            nc.sync.dma_start(xr[:], xres_d[:, m, :])
                            for n in range(SQ // 512):
                                ps = psA.tile([P, 512], f32)
                                for k in range(DC):
                                    nc.tensor.matmul(
                                        ps,
                                        wo_sb[:, k, m * P : (m + 1) * P],
                                        ctxT_sb[:, k, n * 512 : (n + 1) * 512],
                                        start=(k == 0),
                                        stop=(k == DC - 1),
                                    )
                                nc.vector.tensor_add(
                                    yT_sb[:, m, n * 512 : (n + 1) * 512],
                                    ps,
                                    xr[:, n * 512 : (n + 1) * 512],
                                )
                                # bf16(y) into acc_sb: the FFN reads it as its
                                # input activation
                                nc.scalar.activation(
                                    acc_sb[:, m, n * 512 : (n + 1) * 512],
                                    yT_sb[:, m, n * 512 : (n + 1) * 512],
                                    AF.Copy,
                                )

                    # ---------------- Phase D: FFN layer 1 + gelu -----------
                    with tc.tile_pool(name="htp", bufs=1) as htp:
                        hT_sb = htp.tile([P, FC, SQ], bf16)
                        with (
                            tc.tile_pool(name="w2s", bufs=2) as w2s,
                            tc.tile_pool(name="outp", bufs=4) as outp,
                            tc.tile_pool(name="w1s", bufs=3) as w1s,
                        ):
                            w2t0 = w2s.tile([P, FC, P], bf16, tag="w2c", name="w2t0")
                            nc.sync.dma_start(w2t0[:], w2_d[0])
                            for m in range(FC):
                                w1t = w1s.tile([P, DC, P], bf16, tag="w1c")
                                nc.sync.dma_start(w1t[:], w1_d[m])
                                for n in range(SQ // 512):
                                    ps = psA.tile([P, 512], f32)
                                    for k in range(DC):
                                        nc.tensor.matmul(
                                            ps,
                                            w1t[:, k, :],
                                            acc_sb[:, k, n * 512 : (n + 1) * 512],
                                            start=(k == 0),
                                            stop=(k == DC - 1),
                                        )
                                    nc.scalar.activation(
                                        hT_sb[:, m, n * 512 : (n + 1) * 512],
                                        ps,
                                        AF.Gelu,
                                        bias=b1_sb[:, m : m + 1],
                                    )

                            # ------------ Phase E: FFN layer 2 + residual ---
                            for m in range(DC):
                                if m == 0:
                                    w2t = w2t0
                                else:
                                    w2t = w2s.tile([P, FC, P], bf16, tag="w2c")
                                    nc.sync.dma_start(w2t[:], w2_d[m])
                                for n in range(SQ // 512):
                                    ps = psA.tile([P, 512], f32)
                                    for k in range(FC):
                                        nc.tensor.matmul(
                                            ps,
                                            w2t[:, k, :],
                                            hT_sb[:, k, n * 512 : (n + 1) * 512],
                                            start=(k == 0),
                                            stop=(k == FC - 1),
                                        )
                                    ot = outp.tile([P, 512], f32, tag="ot")
                                    nc.scalar.activation(
                                        ot, ps, AF.Identity, bias=b2_sb[:, m : m + 1]
                                    )
                                    nc.vector.tensor_add(
                                        ot, ot, yT_sb[:, m, n * 512 : (n + 1) * 512]
                                    )
                                    nc.sync.dma_start(
                                        outT_d[:, m, n * 512 : (n + 1) * 512], ot
                                    )

    nc.compile()
    return nc


def _get_program():
    if "nc" not in _CACHE:
        _CACHE["nc"] = _build_program()
    return _CACHE["nc"]


def _wlayout(W):
    # [D_in, D_out] -> [P, D_in//P, D_out]
    return np.ascontiguousarray(
        W.reshape(W.shape[0] // P, P, W.shape[1]).transpose(1, 0, 2)
    )


def _blayout(b):
    # [D] -> [P, D//P]
    return np.ascontiguousarray(b.reshape(b.shape[0] // P, P).T)


def prepare_in_maps(x, Wq, bq, Wk, bk, Wv, bv, Wo, bo, W1, b1, W2, b2):
    x = np.asarray(x, np.float32)
    Wq = np.asarray(Wq, np.float32)
    bq = np.asarray(bq, np.float32)
    Wk = np.asarray(Wk, np.float32)
    bk = np.asarray(bk, np.float32)
    Wv = np.asarray(Wv, np.float32)
    bv = np.asarray(bv, np.float32)
    Wo = np.asarray(Wo, np.float32)
    bo = np.asarray(bo, np.float32)
    W1 = np.asarray(W1, np.float32)
    b1 = np.asarray(b1, np.float32)
    W2 = np.asarray(W2, np.float32)
    b2 = np.asarray(b2, np.float32)

    scale = DH ** -0.5
    shared = {
        "wq": _wlayout(Wq * scale).astype(BF16),
        "wk": _wlayout(Wk).astype(BF16),
        "wv": _wlayout(Wv).astype(BF16),
        "wo": _wlayout(Wo).astype(BF16),
        "w1": np.ascontiguousarray(
            W1.reshape(DC, P, FC, P).transpose(2, 1, 0, 3)
        ).astype(BF16),
        "w2": np.ascontiguousarray(
            W2.reshape(FC, P, DC, P).transpose(2, 1, 0, 3)
        ).astype(BF16),
        "bq": _blayout(bq * scale),
        "bk": _blayout(bk),
        "bvb": np.ascontiguousarray(np.broadcast_to(bv, (P, D))).astype(BF16),
        "b1": _blayout(b1),
        "b2": _blayout(b2),
    }

    in_maps = []
    for c in range(NCORES):
        b_idx, half = divmod(c, 2)
        xb = x[b_idx]  # [S, D]
        xbT = xb.T  # [D, S]
        xT = np.ascontiguousarray(
            xbT.reshape(DC, P, S).transpose(1, 0, 2)
        ).astype(BF16)
        xqT = np.ascontiguousarray(
            xbT[:, half * SQ : (half + 1) * SQ]
            .reshape(DC, P, SQ)
            .transpose(1, 0, 2)
        ).astype(BF16)
        xres = np.ascontiguousarray(
            (xbT[:, half * SQ : (half + 1) * SQ] + bo[:, None])
            .reshape(DC, P, SQ)
            .transpose(1, 0, 2)
        ).astype(np.float32)
        in_maps.append(dict(shared, xT=xT, xqT=xqT, xres=xres))
    return in_maps


def assemble_out(results):
    out = np.empty((B, S, D), np.float32)
    for c in range(NCORES):
        b_idx, half = divmod(c, 2)
        outT = results[c]["outT"]  # [P, DC, SQ]
        out[b_idx, half * SQ : (half + 1) * SQ] = (
            outT.transpose(1, 0, 2).reshape(D, SQ).T
        )
    return out


def kernel(**inputs):
    from concourse.bass_utils import run_bass_kernel_spmd

    in_maps = prepare_in_maps(**inputs)
    nc = _get_program()
    res = run_bass_kernel_spmd(nc, in_maps, core_ids=list(range(NCORES)))
    return assemble_out(res.results)


# revision 40
# speedup vs baseline: 1.1993x; 1.1945x over previous
"""Trainium2 Bass kernel for a dense transformer layer (attention + FFN).

Sharding: 8 shards = (batch b, sequence half) pairs. Each core computes the
full K/V projections for its batch (2x redundant) and Q/attention/FFN for its
1024-token query slice. No cross-core communication.

On-device layout is feature-major (transposed): activations live as
[feature, token] so every matmul is lhsT.T @ rhs with natural weight layouts.
Compute in bf16 on the TensorEngine with fp32 PSUM accumulation; residuals in
fp32.
"""

import numpy as np
import ml_dtypes

B, S, D = 4, 2048, 1024
H, DH, F = 16, 64, 4096
P = 128
NCORES = 8
SQ = B * S // NCORES  # 1024 query tokens per core
DC = D // P  # 8 feature chunks
FC = F // P  # 32 ffn chunks
SKC = S // P  # 16 key chunks
NPAIR = H // 2  # 8 head pairs (2 heads per 128-feature chunk)

BF16 = ml_dtypes.bfloat16

_CACHE = {}


def _build_program():
    import concourse.mybir as mybir
    import concourse.tile as tile
    from concourse import bacc

    f32 = mybir.dt.float32
    bf16 = mybir.dt.bfloat16
    AF = mybir.ActivationFunctionType

    nc = bacc.Bacc("TRN2", target_bir_lowering=False, debug=False, num_devices=NCORES)

    xT_d = nc.dram_tensor("xT", [P, DC, S], bf16, kind="ExternalInput")
    xqT_d = nc.dram_tensor("xqT", [P, DC, SQ], bf16, kind="ExternalInput")
    xres_d = nc.dram_tensor("xres", [P, DC, SQ], f32, kind="ExternalInput")
    wq_d = nc.dram_tensor("wq", [P, DC, D], bf16, kind="ExternalInput")
    wk_d = nc.dram_tensor("wk", [P, DC, D], bf16, kind="ExternalInput")
    wv_d = nc.dram_tensor("wv", [P, DC, D], bf16, kind="ExternalInput")
    wo_d = nc.dram_tensor("wo", [P, DC, D], bf16, kind="ExternalInput")
    w1_d = nc.dram_tensor("w1", [FC, P, DC, P], bf16, kind="ExternalInput")
    w2_d = nc.dram_tensor("w2", [DC, P, FC, P], bf16, kind="ExternalInput")
    bq_d = nc.dram_tensor("bq", [P, DC], f32, kind="ExternalInput")
    bk_d = nc.dram_tensor("bk", [P, DC], f32, kind="ExternalInput")
    bvb_d = nc.dram_tensor("bvb", [P, D], bf16, kind="ExternalInput")
    b1_d = nc.dram_tensor("b1", [P, FC], f32, kind="ExternalInput")
    b2_d = nc.dram_tensor("b2", [P, DC], f32, kind="ExternalInput")
    outT_d = nc.dram_tensor("outT", [P, DC, SQ], f32, kind="ExternalOutput")

    with tile.TileContext(nc) as tc:
        with (
            tc.tile_pool(name="psA", bufs=2, space="PSUM") as psA,
            tc.tile_pool(name="psS", bufs=2, space="PSUM") as psS,
            tc.tile_pool(name="psC", bufs=2, space="PSUM") as psC,
            tc.tile_pool(name="biasp", bufs=1) as biasp,
        ):
            bq_sb = biasp.tile([P, DC], f32)
            bk_sb = biasp.tile([P, DC], f32)
            b1_sb = biasp.tile([P, FC], f32)
            b2_sb = biasp.tile([P, DC], f32)
            nc.sync.dma_start(bq_sb[:], bq_d[:])
            nc.sync.dma_start(bk_sb[:], bk_d[:])
            nc.sync.dma_start(b1_sb[:], b1_d[:])
            nc.sync.dma_start(b2_sb[:], b2_d[:])

            with (
                tc.tile_pool(name="ctxp", bufs=1) as ctxp,
                tc.tile_pool(name="wop", bufs=1) as wop,
            ):
                ctxT_sb = ctxp.tile([P, DC, SQ], bf16)
                # attn_out accumulated across pairs (O-projection folded into
                # phase B); later overwritten in place with bf16(y) for the FFN
                acc_sb = ctxp.tile([P, DC, SQ], bf16)
                wo_sb = wop.tile([P, DC, D], bf16)

                # ---------------- Phase A+B: QKV projections + attention ----
                with (
                    tc.tile_pool(name="abp", bufs=1) as abp,
                    tc.tile_pool(name="wvp", bufs=1) as wvp,
                    tc.tile_pool(name="ws", bufs=3) as ws,
                    tc.tile_pool(name="ep", bufs=6) as ep,
                    tc.tile_pool(name="ktp", bufs=2) as ktp,
                    tc.tile_pool(name="qtp", bufs=2) as qtp,
                    tc.tile_pool(name="rp", bufs=2) as rp,
                    tc.tile_pool(name="rbp", bufs=2) as rbp,
                    tc.tile_pool(name="stp", bufs=2) as stp,
                ):
                    # x^T in 4 column-chunk tiles so V/K matmuls start after the
                    # first chunk lands rather than after the full 4MB DMA.
                    xTs = [
                        abp.tile([P, DC, 512], bf16, tag=f"xT{c}", name=f"xT{c}")
                        for c in range(4)
                    ]
                    wvs = [
                        wvp.tile([P, DC, 512], bf16, tag=f"wv{c}", name=f"wv{c}")
                        for c in range(2)
                    ]
                    bvb_sb = abp.tile([P, D], bf16)
                    xqT_sb = abp.tile([P, DC, SQ], bf16)
                    # DMA order matters at startup: the first V matmuls need
                    # xTs[0] + wvs[0] only.
                    # k-chunk interleaved so V's first matmuls start after
                    # ~256KB arrives instead of the full 2MB
                    for k in range(DC):
                        nc.sync.dma_start(
                            xTs[0][:, k : k + 1, :], xT_d[:, k : k + 1, 0:512]
                        )
                        nc.gpsimd.dma_start(
                            wvs[0][:, k : k + 1, :], wv_d[:, k : k + 1, 0:512]
                        )
                    nc.sync.dma_start(bvb_sb[:], bvb_d[:])
                    for c in range(1, 4):
                        nc.sync.dma_start(xTs[c][:], xT_d[:, :, c * 512 : (c + 1) * 512])
                    nc.sync.dma_start(wvs[1][:], wv_d[:, :, 512:1024])
                    nc.sync.dma_start(xqT_sb[:], xqT_d[:])
                    nc.sync.dma_start(wo_sb[:], wo_d[:])

                    # V projection, token-major: v[sk, dv] (+ ones column per head)
                    v_sb = abp.tile([P, SKC, H, DH + 1], bf16)
                    nc.vector.memset(v_sb[:, :, :, DH : DH + 1], 1.0)

                    def emit_v(nv, sks, h0=0, h1=8):
                        nh = h1 - h0
                        for sk in sks:
                            xt = xTs[sk // 4]
                            co = (sk % 4) * P
                            ps = psA.tile([P, 512], f32, tag="ps")
                            for k in range(DC):
                                nc.tensor.matmul(
                                    ps[:, : nh * DH],
                                    xt[:, k, co : co + P],
                                    wvs[nv][:, k, h0 * DH : h1 * DH],
                                    start=(k == 0),
                                    stop=(k == DC - 1),
                                )
                            nc.vector.tensor_add(
                                v_sb[:, sk, nv * 8 + h0 : nv * 8 + h1, 0:DH],
                                ps[:, : nh * DH].rearrange("p (h d) -> p h d", h=nh),
                                bvb_sb[
                                    :, nv * 512 + h0 * DH : nv * 512 + h1 * DH
                                ].rearrange("p (h d) -> p h d", h=nh),
                            )


                    # V(nv=1) matmul chunks, spread across the ACT-bound
                    # attention segments of pairs 1-3 (needed from pair 4 on).
                    V1_SPREAD = {
                        (0, 1): range(0, 2),
                        (1, 0): range(2, 5), (1, 1): range(5, 8),
                        (2, 0): range(8, 11), (2, 1): range(11, 14),
                        (3, 0): range(14, 16),
                    }

                    emit_v(0, range(SKC))
                    for p in range(NPAIR):
                        # K/Q for this pair only; double-buffered so pair p+1's
                        # projections overlap pair p's attention
                        kt = ktp.tile([P, S], bf16, tag="kt", name="kt")
                        qt = qtp.tile([P, SQ], bf16, tag="qt", name="qt")
                        # K projection for feature chunk p (heads 2p, 2p+1)
                        wkt = ws.tile([P, DC, P], bf16, tag="wchunk")
                        nc.sync.dma_start(wkt[:], wk_d[:, :, p * P : (p + 1) * P])
                        for n in range(S // 512):
                            ps = psA.tile([P, 512], f32)
                            for k in range(DC):
                                nc.tensor.matmul(
                                    ps,
                                    wkt[:, k, :],
                                    xTs[n][:, k, :],
                                    start=(k == 0),
                                    stop=(k == DC - 1),
                                )
                            nc.vector.tensor_scalar_add(
                                kt[:, n * 512 : (n + 1) * 512],
                                ps,
                                bk_sb[:, p : p + 1],
                            )
                        # Q projection for feature chunk p (pre-scaled weights)
                        wqt = ws.tile([P, DC, P], bf16, tag="wchunk")
                        nc.sync.dma_start(wqt[:], wq_d[:, :, p * P : (p + 1) * P])
                        for n in range(SQ // 512):
                            ps = psA.tile([P, 512], f32)
                            for k in range(DC):
                                nc.tensor.matmul(
                                    ps,
                                    wqt[:, k, :],
                                    xqT_sb[:, k, n * 512 : (n + 1) * 512],
                                    start=(k == 0),
                                    stop=(k == DC - 1),
                                )
                            nc.vector.tensor_scalar_add(
                                qt[:, n * 512 : (n + 1) * 512],
                                ps,
                                bq_sb[:, p : p + 1],
                            )

                        # Attention for head pair (2p, 2p+1)
                        for sqn in range(SQ // 512):
                            pc0 = psC.tile([P, 512], f32, tag="pc")
                            pc1 = psC.tile([P, 512], f32, tag="pc")
                            for sk in range(SKC):
                                ss = psS.tile([P, 1024], f32)
                                nc.tensor.matmul(
                                    ss[:, 0:512],
                                    kt[0:64, sk * P : (sk + 1) * P],
                                    qt[0:64, sqn * 512 : (sqn + 1) * 512],
                                    start=True,
                                    stop=True,
                                )
                                nc.tensor.matmul(
                                    ss[:, 512:1024],
                                    kt[64:128, sk * P : (sk + 1) * P],
                                    qt[64:128, sqn * 512 : (sqn + 1) * 512],
                                    start=True,
                                    stop=True,
                                )
                                E = ep.tile([P, 1024], bf16)
                                nc.scalar.activation(E, ss, AF.Exp)
                                nc.tensor.matmul(
                                    pc0[:65],
                                    v_sb[:, sk, 2 * p, :],
                                    E[:, 0:512],
                                    start=(sk == 0),
                                    stop=(sk == SKC - 1),
                                )
                                nc.tensor.matmul(
                                    pc1[:65],
                                    v_sb[:, sk, 2 * p + 1, :],
                                    E[:, 512:1024],
                                    start=(sk == 0),
                                    stop=(sk == SKC - 1),
                                )
                            # softmax normalization: ctx / rowsum (approx recip
                            # is ~18 correct bits, plenty for a softmax denom)
                            s0 = rp.tile([1, 512], f32, tag="s")
                            nc.vector.tensor_copy(s0, pc0[64:65, :])
                            r0 = rp.tile([1, 512], f32, tag="r")
                            nc.vector.reciprocal_approx_fast(r0, s0)
                            rb0 = rbp.tile([64, 512], f32, tag="rb")
                            nc.gpsimd.partition_broadcast(rb0, r0)
                            nc.vector.tensor_mul(
                                ctxT_sb[0:64, p, sqn * 512 : (sqn + 1) * 512],
                                pc0[0:64, :],
                                rb0,
                            )
                            s1 = rp.tile([1, 512], f32, tag="s")
                            nc.vector.tensor_copy(s1, pc1[64:65, :])
                            r1 = rp.tile([1, 512], f32, tag="r")
                            nc.vector.reciprocal_approx_fast(r1, s1)
                            rb1 = rbp.tile([64, 512], f32, tag="rb")
                            nc.gpsimd.partition_broadcast(rb1, r1)
                            nc.vector.tensor_mul(
                                ctxT_sb[64:128, p, sqn * 512 : (sqn + 1) * 512],
                                pc1[0:64, :],
                                rb1,
                            )
                            if (p, sqn) in V1_SPREAD:
                                emit_v(1, V1_SPREAD[(p, sqn)])

                # ---------------- Phase C: O projection + residual ----------
                with (
                    tc.tile_pool(name="ytp", bufs=1) as ytp,
                ):
                    yT_sb = ytp.tile([P, DC, SQ], f32)
                    with tc.tile_pool(name="xrp", bufs=3) as xrp:
                        for m in range(DC):
                            xr = xrp.tile([P, SQ], f32, tag="xr")
                            nc.sync.dma_start(xr[:], xres_d[:, m, :])
                            for n in range(SQ // 512):
                                ps = psA.tile([P, 512], f32)
                                for k in range(DC):
                                    nc.tensor.matmul(
                                        ps,
                                        wo_sb[:, k, m * P : (m + 1) * P],
                                        ctxT_sb[:, k, n * 512 : (n + 1) * 512],
                                        start=(k == 0),
                                        stop=(k == DC - 1),
                                    )
                                nc.vector.tensor_add(
                                    yT_sb[:, m, n * 512 : (n + 1) * 512],
                                    ps,
                                    xr[:, n * 512 : (n + 1) * 512],
                                )
                                # bf16(y) into acc_sb: the FFN reads it as its
                                # input activation
                                nc.scalar.activation(
                                    acc_sb[:, m, n * 512 : (n + 1) * 512],
                                    yT_sb[:, m, n * 512 : (n + 1) * 512],
                                    AF.Copy,
                                )

                    # ---------------- Phase D: FFN layer 1 + gelu -----------
                    with tc.tile_pool(name="htp", bufs=1) as htp:
                        hT_sb = htp.tile([P, FC, SQ], bf16)
                        with (
                            tc.tile_pool(name="w2s", bufs=2) as w2s,
                            tc.tile_pool(name="outp", bufs=4) as outp,
                            tc.tile_pool(name="w1s", bufs=3) as w1s,
                        ):
                            w2t0 = w2s.tile([P, FC, P], bf16, tag="w2c", name="w2t0")
                            nc.sync.dma_start(w2t0[:], w2_d[0])
                            for m in range(FC):
                                w1t = w1s.tile([P, DC, P], bf16, tag="w1c")
                                nc.sync.dma_start(w1t[:], w1_d[m])
                                for n in range(SQ // 512):
                                    ps = psA.tile([P, 512], f32)
                                    for k in range(DC):
                                        nc.tensor.matmul(
                                            ps,
                                            w1t[:, k, :],
                                            acc_sb[:, k, n * 512 : (n + 1) * 512],
                                            start=(k == 0),
                                            stop=(k == DC - 1),
                                        )
                                    nc.scalar.activation(
                                        hT_sb[:, m, n * 512 : (n + 1) * 512],
                                        ps,
                                        AF.Gelu,
                                        bias=b1_sb[:, m : m + 1],
                                    )

                            # ------------ Phase E: FFN layer 2 + residual ---
                            for m in range(DC):
                                if m == 0:
                                    w2t = w2t0
                                else:
                                    w2t = w2s.tile([P, FC, P], bf16, tag="w2c")
                                    nc.sync.dma_start(w2t[:], w2_d[m])
                                for n in range(SQ // 512):
                                    ps = psA.tile([P, 512], f32)
                                    for k in range(FC):
                                        nc.tensor.matmul(
                                            ps,
                                            w2t[:, k, :],
                                            hT_sb[:, k, n * 512 : (n + 1) * 512],
                                            start=(k == 0),
                                            stop=(k == FC - 1),
                                        )
                                    ot = outp.tile([P, 512], f32, tag="ot")
                                    nc.scalar.activation(
                                        ot, ps, AF.Identity, bias=b2_sb[:, m : m + 1]
                                    )
                                    nc.vector.tensor_add(
                                        ot, ot, yT_sb[:, m, n * 512 : (n + 1) * 512]
                                    )
                                    nc.sync.dma_start(
                                        outT_d[:, m, n * 512 : (n + 1) * 512], ot
                                    )

    nc.compile()
    return nc


def _get_program():
    if "nc" not in _CACHE:
        _CACHE["nc"] = _build_program()
    return _CACHE["nc"]


def _wlayout(W):
    # [D_in, D_out] -> [P, D_in//P, D_out]
    return np.ascontiguousarray(
        W.reshape(W.shape[0] // P, P, W.shape[1]).transpose(1, 0, 2)
    )


def _blayout(b):
    # [D] -> [P, D//P]
    return np.ascontiguousarray(b.reshape(b.shape[0] // P, P).T)


def prepare_in_maps(x, Wq, bq, Wk, bk, Wv, bv, Wo, bo, W1, b1, W2, b2):
    x = np.asarray(x, np.float32)
    Wq = np.asarray(Wq, np.float32)
    bq = np.asarray(bq, np.float32)
    Wk = np.asarray(Wk, np.float32)
    bk = np.asarray(bk, np.float32)
    Wv = np.asarray(Wv, np.float32)
    bv = np.asarray(bv, np.float32)
    Wo = np.asarray(Wo, np.float32)
    bo = np.asarray(bo, np.float32)
    W1 = np.asarray(W1, np.float32)
    b1 = np.asarray(b1, np.float32)
    W2 = np.asarray(W2, np.float32)
    b2 = np.asarray(b2, np.float32)

    scale = DH ** -0.5
    shared = {
        "wq": _wlayout(Wq * scale).astype(BF16),
        "wk": _wlayout(Wk).astype(BF16),
        "wv": _wlayout(Wv).astype(BF16),
        "wo": _wlayout(Wo).astype(BF16),
        "w1": np.ascontiguousarray(
            W1.reshape(DC, P, FC, P).transpose(2, 1, 0, 3)
        ).astype(BF16),
        "w2": np.ascontiguousarray(
            W2.reshape(FC, P, DC, P).transpose(2, 1, 0, 3)
        ).astype(BF16),
        "bq": _blayout(bq * scale),
        "bk": _blayout(bk),
        "bvb": np.ascontiguousarray(np.broadcast_to(bv, (P, D))).astype(BF16),
        "b1": _blayout(b1),
        "b2": _blayout(b2),
    }

    in_maps = []
    for c in range(NCORES):
        b_idx, half = divmod(c, 2)
        xb = x[b_idx]  # [S, D]
        xbT = xb.T  # [D, S]
        xT = np.ascontiguousarray(
            xbT.reshape(DC, P, S).transpose(1, 0, 2)
        ).astype(BF16)
        xqT = np.ascontiguousarray(
            xbT[:, half * SQ : (half + 1) * SQ]
            .reshape(DC, P, SQ)
            .transpose(1, 0, 2)
        ).astype(BF16)
        xres = np.ascontiguousarray(
            (xbT[:, half * SQ : (half + 1) * SQ] + bo[:, None])
            .reshape(DC, P, SQ)
            .transpose(1, 0, 2)
        ).astype(np.float32)
        in_maps.append(dict(shared, xT=xT, xqT=xqT, xres=xres))
    return in_maps


def assemble_out(results):
    out = np.empty((B, S, D), np.float32)
    for c in range(NCORES):
        b_idx, half = divmod(c, 2)
        outT = results[c]["outT"]  # [P, DC, SQ]
        out[b_idx, half * SQ : (half + 1) * SQ] = (
            outT.transpose(1, 0, 2).reshape(D, SQ).T
        )
    return out


def kernel(**inputs):
    from concourse.bass_utils import run_bass_kernel_spmd

    in_maps = prepare_in_maps(**inputs)
    nc = _get_program()
    res = run_bass_kernel_spmd(nc, in_maps, core_ids=list(range(NCORES)))
    return assemble_out(res.results)
